# revision 32
# baseline (speedup 1.0000x reference)
"""AttnBlock3D (GroupNorm + single-head self-attention over 4096 voxels + residual)
for Trainium2, SPMD over 8 NeuronCores.

2D sharding: core = b*4 + q*2 + kk  (b batch, q query-half, kk key-half).

Host: GroupNorm (fp64 stats) and all bias folding happen on the host; each core
receives its 2048-column query half and key half of normalized h in fp8(e4m3),
plus fp8 Q/K/V weights prescaled by WS=16 and a bf16 output-projection weight.
K and V biases fold out exactly (k-bias is constant across keys so it cancels
in softmax; v-bias times sum-of-probs folds into the host-side constant
bo_eff = bo + wo @ bv).  Only bq remains on-device (scaled by WS).

Device (one NEFF, no collectives): all of Q/K/V projection, logits and A@V run
as fp8 DoubleRow matmuls (256-deep contraction per instruction); probs are fp8
via exp(S*scale - CSH) on ScalarE (the constant shift cancels in softmax and
keeps fp8 probs < 240); the row-sum l rides a fp8 ones-matmul; out-projection
wo @ o_u runs in bf16; unnormalized F_u (bf16) and l (f32) go back to the host.
Host combine: out = x + (F_u^k0 + F_u^k1) / (WS * (l^k0 + l^k1)) + bo_eff.
"""

import sys

if "/opt/trn_rl_repo" not in sys.path:
    sys.path.insert(0, "/opt/trn_rl_repo")

import numpy as np

P = 128
C = 512
CO = C // P          # 4 channel chunks
CP = CO // 2         # 2 channel-chunk pairs (DoubleRow)
N = 4096             # spatial size (16^3)
NH = N // 2          # 2048 local keys / queries
KBLK = NH // 512     # 4 key blocks
ITQ = NH // 512      # 4 query slabs
JCK = NH // P        # 16 local key chunks
JPR = JCK // 2       # 8 key chunk pairs (DoubleRow)
G = 32               # groups
GS = C // G          # 16 channels per group
EPS = 1e-6
WS = 16.0            # fp8 weight prescale (folded into exp scale + host combine)
CSH = 5.0            # exp shift: probs = exp(S*scale - CSH), cancels in softmax
SM_SCALE = float(C) ** -0.5 / (WS * WS)

_CACHE = {}


def _build_program():
    import concourse.bass as bass
    import concourse.tile as tile
    import concourse.mybir as mybir
    from concourse import bacc
    from contextlib import ExitStack

    f32 = mybir.dt.float32
    bf16 = mybir.dt.bfloat16
    f8 = mybir.dt.float8e4
    AF = mybir.ActivationFunctionType
    DR = mybir.MatmulPerfMode.DoubleRow

    nc = bacc.Bacc("TRN2", target_bir_lowering=False)

    hkv = nc.dram_tensor("hkv", [P, KBLK, CO, 512], f8, kind="ExternalInput")
    hq = nc.dram_tensor("hq", [P, ITQ, CO, 512], f8, kind="ExternalInput")
    wk8 = nc.dram_tensor("wk8", [P, CO, C], f8, kind="ExternalInput")
    wv8 = nc.dram_tensor("wv8", [P, CO, C], f8, kind="ExternalInput")
    wq8 = nc.dram_tensor("wq8", [P, CO, C], f8, kind="ExternalInput")
    wot = nc.dram_tensor("wot", [P, CO, C], f8, kind="ExternalInput")
    bqb = nc.dram_tensor("bqb", [P, CO], f32, kind="ExternalInput")
    out = nc.dram_tensor("out", [P, ITQ, CO, 512], bf16, kind="ExternalOutput")
    lout = nc.dram_tensor("lout", [ITQ, 512], f32, kind="ExternalOutput")
    warm = nc.dram_tensor("warm", [1, 1], f32, kind="ExternalOutput")

    with ExitStack() as ctx:
        tc = ctx.enter_context(tile.TileContext(nc))
        big = ctx.enter_context(tc.tile_pool(name="big", bufs=1))
        wrk = ctx.enter_context(tc.tile_pool(name="wrk", bufs=3))
        fpl = ctx.enter_context(tc.tile_pool(name="fpl", bufs=2))
        psA = ctx.enter_context(tc.tile_pool(name="psA", bufs=3, space="PSUM"))
        psO = ctx.enter_context(tc.tile_pool(name="psO", bufs=4, space="PSUM"))
        psS = ctx.enter_context(tc.tile_pool(name="psS", bufs=1, space="PSUM"))

        # ---- persistent SBUF tiles -------------------------------------
        Hkv = big.tile([P, KBLK, CO, 512], f8)    # key-half h
        Hq = big.tile([P, ITQ, CO, 512], f8)      # query-half h
        Kt = big.tile([P, CO, NH], f8)            # k16[c, j]
        Qt = big.tile([P, CO, NH], f8)            # q16[c, i]
        VT = big.tile([P, JCK, C], f8)            # v16^T[j, c]
        Ot = big.tile([P, CO, NH], f8)            # o_u (true scale)
        wk_s = big.tile([P, CO, C], f8)
        wv_s = big.tile([P, CO, C], f8)
        wq_s = big.tile([P, CO, C], f8)
        wo_s = big.tile([P, CO, C], f8)
        bq_s = big.tile([P, CO], f32)
        ones8 = big.tile([P, 2, P], f8)
        ones_bf = big.tile([P, P], bf16)
        junk_rhs = big.tile([P, 2, 512], f8)
        csh_s = big.tile([P, 1], f32)
        nc.vector.memset(ones8, 1.0)
        nc.gpsimd.memset(ones_bf, 1.0)
        nc.gpsimd.memset(junk_rhs, 0.0)
        nc.vector.memset(csh_s, -CSH)

        # PE warm-up while input DMAs stream (opens the clock gate and
        # exercises the fp8 DoubleRow path). The accumulation chain plus the
        # DMA'd sample keep walrus from dead-code-eliminating it.
        junk_ps = psS.tile([P, 512], f32, tag="l", name="junk_ps")
        NWARM = 12
        for j in range(NWARM):
            nc.tensor.matmul(
                junk_ps[:, 0:256], ones8[:], junk_rhs[:, :, 0:256],
                start=(j == 0), stop=(j == NWARM - 1), perf_mode=DR,
            )
        wrm = big.tile([1, 1], f32)
        nc.vector.tensor_copy(wrm[:], junk_ps[0:1, 0:1])
        nc.sync.dma_start(warm[:, :], wrm[:])

        # ---- input DMAs: interleaved across the 3 DMA-capable queues in
        # consumption order (K path first, then Q path, then V/O weights) so
        # arrival order under shared HBM bandwidth matches the compute order.
        nc.sync.dma_start(wk_s[:, 0:2, :], wk8[:, 0:2, :])
        nc.gpsimd.dma_start(wk_s[:, 2:4, :], wk8[:, 2:4, :])
        nc.scalar.dma_start(Hkv[:, 0, :, :], hkv[:, 0, :, :])
        nc.sync.dma_start(Hkv[:, 1, :, :], hkv[:, 1, :, :])
        nc.gpsimd.dma_start(Hkv[:, 2, :, :], hkv[:, 2, :, :])
        nc.scalar.dma_start(Hkv[:, 3, :, :], hkv[:, 3, :, :])
        nc.sync.dma_start(wq_s[:, 0:2, :], wq8[:, 0:2, :])
        nc.gpsimd.dma_start(wq_s[:, 2:4, :], wq8[:, 2:4, :])
        nc.scalar.dma_start(bq_s[:], bqb[:, :])
        nc.sync.dma_start(Hq[:, 0, :, :], hq[:, 0, :, :])
        nc.gpsimd.dma_start(Hq[:, 1, :, :], hq[:, 1, :, :])
        nc.scalar.dma_start(Hq[:, 2, :, :], hq[:, 2, :, :])
        nc.sync.dma_start(Hq[:, 3, :, :], hq[:, 3, :, :])
        nc.gpsimd.dma_start(wv_s[:], wv8[:, :, :])
        nc.scalar.dma_start(wo_s[:], wot[:, :, :])

        # ---- projections (fp8 DoubleRow: 256-deep contraction) ----------
        # K: k16[cc, blk] = sum_ci wk16[cc, ci] h[ci, blk]
        for blk in range(KBLK):
            for cc in range(CO):
                ps = psA.tile([P, 512], f32, tag="mm", name=f"psk_{blk}_{cc}")
                for cp in range(CP):
                    nc.tensor.matmul(
                        ps[:],
                        wk_s[:, 2 * cp:2 * cp + 2, cc * P:(cc + 1) * P],
                        Hkv[:, blk, 2 * cp:2 * cp + 2, :],
                        start=(cp == 0), stop=(cp == CP - 1), perf_mode=DR,
                    )
                if cc % 2 == 0:
                    nc.vector.tensor_copy(
                        Kt[:, cc, blk * 512:(blk + 1) * 512], ps[:]
                    )
                else:
                    nc.scalar.copy(Kt[:, cc, blk * 512:(blk + 1) * 512], ps[:])

        # Q: q16[cc, it] = sum_ci wq16[cc, ci] hq[ci, it] + 16*bq
        for it in range(ITQ):
            for cc in range(CO):
                ps = psA.tile([P, 512], f32, tag="mm", name=f"psq_{it}_{cc}")
                for cp in range(CP):
                    nc.tensor.matmul(
                        ps[:],
                        wq_s[:, 2 * cp:2 * cp + 2, cc * P:(cc + 1) * P],
                        Hq[:, it, 2 * cp:2 * cp + 2, :],
                        start=(cp == 0), stop=(cp == CP - 1), perf_mode=DR,
                    )
                if cc % 2 == 0:
                    nc.vector.tensor_scalar_add(
                        Qt[:, cc, it * 512:(it + 1) * 512], ps[:],
                        bq_s[:, cc:cc + 1],
                    )
                else:
                    nc.scalar.activation(
                        out=Qt[:, cc, it * 512:(it + 1) * 512], in_=ps[:],
                        func=AF.Identity, bias=bq_s[:, cc:cc + 1], scale=1.0,
                    )

        # V^T: vt[jc, c] = sum_ci h[ci, jc]^T wv[ci, c]
        for jc in range(JCK):
            ps = psA.tile([P, 512], f32, tag="mm", name=f"psv_{jc}")
            for cp in range(CP):
                nc.tensor.matmul(
                    ps[:],
                    Hkv[:, jc // 4, 2 * cp:2 * cp + 2, (jc % 4) * P:(jc % 4 + 1) * P],
                    wv_s[:, 2 * cp:2 * cp + 2, :],
                    start=(cp == 0), stop=(cp == CP - 1), perf_mode=DR,
                )
            if jc % 2 == 0:
                nc.vector.tensor_copy(VT[:, jc, :], ps[:])
            else:
                nc.scalar.copy(VT[:, jc, :], ps[:])

        # ---- attention + fused output projection -------------------------
        def emit_final(it, half, endgame=False):
            for cc in (half, half + 2):
                ps = psA.tile([P, 512], f32, tag="mm", name=f"psf_{it}_{cc}")
                for cp in range(CP):
                    nc.tensor.matmul(
                        ps[:],
                        wo_s[:, 2 * cp:2 * cp + 2, cc * P:(cc + 1) * P],
                        Ot[:, 2 * cp:2 * cp + 2, it * 512:(it + 1) * 512],
                        start=(cp == 0), stop=(cp == CP - 1), perf_mode=DR,
                    )
                ft = fpl.tile([P, 512], bf16, tag="f", name=f"ft_{it}_{cc}")
                if endgame and cc >= 2:
                    nc.scalar.copy(ft[:], ps[:])
                else:
                    nc.vector.tensor_copy(ft[:], ps[:])
                eng = nc.sync if cc % 2 == 0 else nc.gpsimd
                eng.dma_start(out[:, it, cc, :], ft[:])

        pend_evac = None
        for it in range(ITQ):
            o_ps = [
                psO.tile([P, 512], f32, tag="o", name=f"o_ps_{it}_{cc}")
                for cc in range(CO)
            ]
            # row-sum l rides GpSimd: accumulate fp8 prob tiles into f32,
            # cast to bf16, one ones-matmul per slab (in the next slab's evac)
            pts = wrk.tile([P, 512], f32, tag="pts", bufs=2, name=f"pts_{it}")
            ptsb = wrk.tile([P, 512], bf16, tag="ptsb", bufs=2,
                            name=f"ptsb_{it}")

            def emit_avl(jp, pt, o_ps=o_ps, pts=pts):
                if jp == 0:
                    nc.gpsimd.tensor_copy(pts[:], pt[:, 0, :])
                else:
                    nc.gpsimd.tensor_add(pts[:], pts[:], pt[:, 0, :])
                nc.gpsimd.tensor_add(pts[:], pts[:], pt[:, 1, :])
                for cc in range(CO):
                    nc.tensor.matmul(
                        o_ps[cc][:],
                        VT[:, 2 * jp:2 * jp + 2, cc * P:(cc + 1) * P],
                        pt[:],
                        start=(jp == 0), stop=(jp == JPR - 1), perf_mode=DR,
                    )

            # AV lags 2 pairs at slab start (hides prev-slab PSUM evac), 1 after
            prev = []
            for jp in range(JPR):
                if jp == 0 and pend_evac is not None:
                    pend_evac()     # prev slab's PSUM evac, first thing
                if jp == 3 and it > 0:
                    emit_final(it - 1, 0)   # overlap prev slab's out-proj
                if jp == 5 and it > 0:
                    emit_final(it - 1, 1)
                pt = wrk.tile(
                    [P, 2, 512], f8, tag="pt", bufs=4, name=f"pt_{it}_{jp}"
                )
                for t in range(2):
                    jc = 2 * jp + t
                    st = psA.tile([P, 512], f32, tag="mm", name=f"st_{it}_{jc}")
                    for cp in range(CP):
                        nc.tensor.matmul(
                            st[:],
                            Kt[:, 2 * cp:2 * cp + 2, jc * P:(jc + 1) * P],
                            Qt[:, 2 * cp:2 * cp + 2, it * 512:(it + 1) * 512],
                            start=(cp == 0), stop=(cp == CP - 1), perf_mode=DR,
                        )
                    nc.scalar.activation(
                        out=pt[:, t, :], in_=st[:], func=AF.Exp,
                        bias=csh_s[:], scale=SM_SCALE,
                    )
                prev.append((jp, pt))
                lag = 2 if jp < 4 else 1
                while len(prev) > lag:
                    emit_avl(*prev.pop(0))
            for pr in prev:
                emit_avl(*pr)
            nc.gpsimd.tensor_copy(ptsb[:], pts[:])

            def _evac(it=it, o_ps=o_ps, ptsb=ptsb, endgame=(it == ITQ - 1)):
                l_ps = psS.tile([P, 512], f32, tag="l", name=f"l_ps_{it}")
                nc.tensor.matmul(
                    l_ps[:], ones_bf[:], ptsb[:], start=True, stop=True,
                )
                lt = wrk.tile([1, 512], f32, tag="lt", name=f"lt_{it}")
                nc.vector.tensor_copy(lt[:], l_ps[0:1, :])
                nc.sync.dma_start(lout[it:it + 1, :], lt[:])
                for cc in range(CO):
                    if endgame and cc % 2 == 1:
                        nc.scalar.copy(
                            Ot[:, cc, it * 512:(it + 1) * 512], o_ps[cc][:]
                        )
                    else:
                        nc.vector.tensor_copy(
                            Ot[:, cc, it * 512:(it + 1) * 512], o_ps[cc][:]
                        )
            pend_evac = _evac
        pend_evac()
        emit_final(ITQ - 1, 0, endgame=True)
        emit_final(ITQ - 1, 1, endgame=True)

    nc.compile()
    return nc


def _get_program():
    if "nc" not in _CACHE:
        _CACHE["nc"] = _build_program()
    return _CACHE["nc"]


def _tile_cp(a, dtype):
    """[C, M] -> [P, CO, M] with c = co*128 + p."""
    m = a.shape[1]
    return np.ascontiguousarray(
        a.reshape(CO, P, m).transpose(1, 0, 2).astype(dtype)
    )


def _tile_c(v):
    """[C] -> [P, CO] with c = co*128 + p."""
    return np.ascontiguousarray(v.reshape(CO, P).T, dtype=np.float32)


def _blockmajor(xt, nblk):
    """[P, CO, nblk*512] -> [P, nblk, CO, 512] contiguous."""
    return np.ascontiguousarray(
        xt.reshape(P, CO, nblk, 512).transpose(0, 2, 1, 3)
    )


def _host_prep(x, gamma, beta, wq, bq, wk, bk, wv, bv, wo, bo):
    import ml_dtypes

    bf16 = ml_dtypes.bfloat16
    f8 = ml_dtypes.float8_e4m3
    x = np.asarray(x, dtype=np.float32)
    b = x.shape[0]
    xv = x.reshape(b, C, N)

    # host GroupNorm (fp64 stats, f32 apply)
    gamma = np.asarray(gamma, np.float32)
    beta = np.asarray(beta, np.float32)
    xg = xv.reshape(b, G, GS * N)
    mean = xg.mean(axis=2, dtype=np.float64)                 # [b, G]
    var = xg.var(axis=2, dtype=np.float64)                   # [b, G]
    rstd = 1.0 / np.sqrt(var + EPS)
    mean_c = np.repeat(mean, GS, axis=1).astype(np.float32)  # [b, C]
    rstd_c = np.repeat(rstd, GS, axis=1).astype(np.float32)
    scl = rstd_c * gamma[None, :]
    shf = beta[None, :] - scl * mean_c
    h = xv * scl[:, :, None] + shf[:, :, None]               # [b, C, N] f32

    wqT = np.ascontiguousarray(np.asarray(wq, np.float32).T) * WS
    wkT = np.ascontiguousarray(np.asarray(wk, np.float32).T) * WS
    wvT = np.ascontiguousarray(np.asarray(wv, np.float32).T)
    woT = np.ascontiguousarray(np.asarray(wo, np.float32).T) * WS

    wq_t = _tile_cp(wqT, f8)
    wk_t = _tile_cp(wkT, f8)
    wv_t = _tile_cp(wvT, f8)
    wo_t = _tile_cp(woT, f8)
    bq_t = _tile_c(np.asarray(bq, np.float32) * WS)

    halves = {}
    for bi in range(b):
        ht = _tile_cp(h[bi], f8)                             # [P, CO, N]
        for hf in range(2):
            halves[(bi, hf)] = _blockmajor(
                ht[:, :, hf * NH:(hf + 1) * NH], KBLK
            )

    in_maps = []
    for core in range(8):
        bi, qh, kk = core // 4, (core // 2) % 2, core % 2
        in_maps.append({
            "hkv": halves[(bi, kk)], "hq": halves[(bi, qh)],
            "wk8": wk_t, "wv8": wv_t, "wq8": wq_t, "wot": wo_t,
            "bqb": bq_t,
        })
    return in_maps, b


def kernel(x, gamma, beta, wq, bq, wk, bk, wv, bv, wo, bo):
    from concourse.bass_utils import run_bass_kernel_spmd

    nc = _get_program()
    in_maps, b = _host_prep(x, gamma, beta, wq, bq, wk, bk, wv, bv, wo, bo)
    res = run_bass_kernel_spmd(nc, in_maps, core_ids=list(range(8)))

    x = np.asarray(x, dtype=np.float32)
    xv = x.reshape(b, C, N)
    bo_eff = (
        np.asarray(bo, np.float64)
        + np.asarray(wo, np.float64) @ np.asarray(bv, np.float64)
    )
    outp = np.empty((b, C, N), dtype=np.float32)
    for bi in range(b):
        for qh in range(2):
            ca = bi * 4 + qh * 2 + 0   # key-half 0
            cb = bi * 4 + qh * 2 + 1   # key-half 1
            fu = (
                res.results[ca]["out"].astype(np.float64)
                + res.results[cb]["out"].astype(np.float64)
            )  # [P, ITQ, CO, 512]
            l = (
                res.results[ca]["lout"].astype(np.float64)
                + res.results[cb]["lout"].astype(np.float64)
            ).reshape(NH)
            fu = fu.transpose(2, 0, 1, 3).reshape(C, NH)  # channel-major
            cols = slice(qh * NH, (qh + 1) * NH)
            outp[bi, :, cols] = (
                xv[bi][:, cols] + fu / (WS * l[None, :]) + bo_eff[:, None]
            )
    return outp.reshape(b, C, 16, 16, 16)


# revision 37
# speedup vs baseline: 1.1893x; 1.1893x over previous
"""AttnBlock3D (GroupNorm + single-head self-attention over 4096 voxels + residual)
for Trainium2, SPMD over 8 NeuronCores.

2D sharding: core = b*4 + q*2 + kk  (b batch, q query-half, kk key-half).

Host: GroupNorm (fp64 stats) and all bias folding happen on the host; each core
receives its 2048-column query half and key half of normalized h in fp8(e4m3),
plus fp8 Q/K/V weights prescaled by WS=16 and a bf16 output-projection weight.
K and V biases fold out exactly (k-bias is constant across keys so it cancels
in softmax; v-bias times sum-of-probs folds into the host-side constant
bo_eff = bo + wo @ bv).  Only bq remains on-device (scaled by WS).

Device (one NEFF, no collectives): all of Q/K/V projection, logits and A@V run
as fp8 DoubleRow matmuls (256-deep contraction per instruction); probs are fp8
via exp(S*scale - CSH) on ScalarE (the constant shift cancels in softmax and
keeps fp8 probs < 240); the row-sum l rides a fp8 ones-matmul; out-projection
wo @ o_u runs in bf16; unnormalized F_u (bf16) and l (f32) go back to the host.
Host combine: out = x + (F_u^k0 + F_u^k1) / (WS * (l^k0 + l^k1)) + bo_eff.
"""

import sys

if "/opt/trn_rl_repo" not in sys.path:
    sys.path.insert(0, "/opt/trn_rl_repo")

import numpy as np

P = 128
C = 512
CO = C // P          # 4 channel chunks
CP = CO // 2         # 2 channel-chunk pairs (DoubleRow)
N = 4096             # spatial size (16^3)
NH = N // 2          # 2048 local keys / queries
KBLK = NH // 512     # 4 key blocks
ITQ = NH // 512      # 4 query slabs
JCK = NH // P        # 16 local key chunks
JPR = JCK // 2       # 8 key chunk pairs (DoubleRow)
G = 32               # groups
GS = C // G          # 16 channels per group
EPS = 1e-6
WS = 16.0            # fp8 weight prescale (folded into exp scale + host combine)
CSH = 5.0            # exp shift: probs = exp(S*scale - CSH), cancels in softmax
SM_SCALE = float(C) ** -0.5 / (WS * WS)

_CACHE = {}


def _build_program():
    import concourse.bass as bass
    import concourse.tile as tile
    import concourse.mybir as mybir
    from concourse import bacc
    from contextlib import ExitStack

    f32 = mybir.dt.float32
    bf16 = mybir.dt.bfloat16
    f8 = mybir.dt.float8e4
    AF = mybir.ActivationFunctionType
    DR = mybir.MatmulPerfMode.DoubleRow

    nc = bacc.Bacc("TRN2", target_bir_lowering=False)

    hkv = nc.dram_tensor("hkv", [P, KBLK, CO, 512], f8, kind="ExternalInput")
    hq = nc.dram_tensor("hq", [P, ITQ, CO, 512], f8, kind="ExternalInput")
    wk8 = nc.dram_tensor("wk8", [P, CO, C], f8, kind="ExternalInput")
    wv8 = nc.dram_tensor("wv8", [P, CO, C], f8, kind="ExternalInput")
    wq8 = nc.dram_tensor("wq8", [P, CO, C], f8, kind="ExternalInput")
    wot = nc.dram_tensor("wot", [P, CO, C], f8, kind="ExternalInput")
    bqb = nc.dram_tensor("bqb", [P, CO], f32, kind="ExternalInput")
    out = nc.dram_tensor("out", [P, ITQ, CO, 512], bf16, kind="ExternalOutput")
    lout = nc.dram_tensor("lout", [ITQ, 512], f32, kind="ExternalOutput")
    warm = nc.dram_tensor("warm", [1, 1], f32, kind="ExternalOutput")

    with ExitStack() as ctx:
        tc = ctx.enter_context(tile.TileContext(nc))
        big = ctx.enter_context(tc.tile_pool(name="big", bufs=1))
        wrk = ctx.enter_context(tc.tile_pool(name="wrk", bufs=3))
        fpl = ctx.enter_context(tc.tile_pool(name="fpl", bufs=2))
        psA = ctx.enter_context(tc.tile_pool(name="psA", bufs=3, space="PSUM"))
        psO = ctx.enter_context(tc.tile_pool(name="psO", bufs=4, space="PSUM"))
        psS = ctx.enter_context(tc.tile_pool(name="psS", bufs=1, space="PSUM"))

        # ---- persistent SBUF tiles -------------------------------------
        Hkv = big.tile([P, KBLK, CO, 512], f8)    # key-half h
        Hq = big.tile([P, ITQ, CO, 512], f8)      # query-half h
        Kt = big.tile([P, CO, NH], f8)            # k16[c, j]
        Qt = big.tile([P, CO, NH], f8)            # q16[c, i]
        VT = big.tile([P, JCK, C], f8)            # v16^T[j, c]
        Ot = big.tile([P, CO, NH], f8)            # o_u (true scale)
        wk_s = big.tile([P, CO, C], f8)
        wv_s = big.tile([P, CO, C], f8)
        wq_s = big.tile([P, CO, C], f8)
        wo_s = big.tile([P, CO, C], f8)
        bq_s = big.tile([P, CO], f32)
        ones8 = big.tile([P, 2, P], f8)
        junk_rhs = big.tile([P, 2, 512], f8)
        csh_s = big.tile([P, 1], f32)
        nc.vector.memset(ones8, 1.0)
        nc.gpsimd.memset(junk_rhs, 0.0)
        nc.vector.memset(csh_s, -CSH)

        # PE warm-up while input DMAs stream (opens the clock gate and
        # exercises the fp8 DoubleRow path). The accumulation chain plus the
        # DMA'd sample keep walrus from dead-code-eliminating it.
        junk_ps = psS.tile([P, 512], f32, tag="l", name="junk_ps")
        NWARM = 12
        for j in range(NWARM):
            sl = slice((j % 2) * 256, (j % 2) * 256 + 256)
            nc.tensor.matmul(
                junk_ps[0:32, sl], ones8[:, :, (j % 4) * 32:(j % 4) * 32 + 32],
                junk_rhs[:, :, sl],
                start=(j < 2), stop=(j >= NWARM - 2), perf_mode=DR,
            )
        wrm = big.tile([1, 1], f32)
        nc.vector.tensor_copy(wrm[:], junk_ps[0:1, 0:1])
        nc.sync.dma_start(warm[:, :], wrm[:])

        # ---- input DMAs: interleaved across the 3 DMA-capable queues in
        # consumption order (K path first, then Q path, then V/O weights) so
        # arrival order under shared HBM bandwidth matches the compute order.
        nc.sync.dma_start(wk_s[:, 0:2, :], wk8[:, 0:2, :])
        nc.gpsimd.dma_start(wk_s[:, 2:4, :], wk8[:, 2:4, :])
        nc.scalar.dma_start(Hkv[:, 0, :, :], hkv[:, 0, :, :])
        nc.sync.dma_start(Hkv[:, 1, :, :], hkv[:, 1, :, :])
        nc.gpsimd.dma_start(Hkv[:, 2, :, :], hkv[:, 2, :, :])
        nc.scalar.dma_start(Hkv[:, 3, :, :], hkv[:, 3, :, :])
        nc.sync.dma_start(wq_s[:, 0:2, :], wq8[:, 0:2, :])
        nc.gpsimd.dma_start(wq_s[:, 2:4, :], wq8[:, 2:4, :])
        nc.scalar.dma_start(bq_s[:], bqb[:, :])
        nc.sync.dma_start(Hq[:, 0, :, :], hq[:, 0, :, :])
        nc.gpsimd.dma_start(Hq[:, 1, :, :], hq[:, 1, :, :])
        nc.scalar.dma_start(Hq[:, 2, :, :], hq[:, 2, :, :])
        nc.sync.dma_start(Hq[:, 3, :, :], hq[:, 3, :, :])
        nc.gpsimd.dma_start(wv_s[:], wv8[:, :, :])
        nc.scalar.dma_start(wo_s[:], wot[:, :, :])

        # ---- projections (fp8 DoubleRow: 256-deep contraction) ----------
        # K: k16[cc, blk] = sum_ci wk16[cc, ci] h[ci, blk]
        for blk in range(KBLK):
            for cc in range(CO):
                ps = psA.tile([P, 512], f32, tag="mm", name=f"psk_{blk}_{cc}")
                for cp in range(CP):
                    nc.tensor.matmul(
                        ps[:],
                        wk_s[:, 2 * cp:2 * cp + 2, cc * P:(cc + 1) * P],
                        Hkv[:, blk, 2 * cp:2 * cp + 2, :],
                        start=(cp == 0), stop=(cp == CP - 1), perf_mode=DR,
                    )
                if cc % 2 == 0:
                    nc.vector.tensor_copy(
                        Kt[:, cc, blk * 512:(blk + 1) * 512], ps[:]
                    )
                else:
                    nc.scalar.copy(Kt[:, cc, blk * 512:(blk + 1) * 512], ps[:])

        # Q: q16[cc, it] = sum_ci wq16[cc, ci] hq[ci, it] + 16*bq
        for it in range(ITQ):
            for cc in range(CO):
                ps = psA.tile([P, 512], f32, tag="mm", name=f"psq_{it}_{cc}")
                for cp in range(CP):
                    nc.tensor.matmul(
                        ps[:],
                        wq_s[:, 2 * cp:2 * cp + 2, cc * P:(cc + 1) * P],
                        Hq[:, it, 2 * cp:2 * cp + 2, :],
                        start=(cp == 0), stop=(cp == CP - 1), perf_mode=DR,
                    )
                if cc % 2 == 0:
                    nc.vector.tensor_scalar_add(
                        Qt[:, cc, it * 512:(it + 1) * 512], ps[:],
                        bq_s[:, cc:cc + 1],
                    )
                else:
                    nc.scalar.activation(
                        out=Qt[:, cc, it * 512:(it + 1) * 512], in_=ps[:],
                        func=AF.Identity, bias=bq_s[:, cc:cc + 1], scale=1.0,
                    )

        # V^T: vt[jc, c] = sum_ci h[ci, jc]^T wv[ci, c]
        for jc in range(JCK):
            ps = psA.tile([P, 512], f32, tag="mm", name=f"psv_{jc}")
            for cp in range(CP):
                nc.tensor.matmul(
                    ps[:],
                    Hkv[:, jc // 4, 2 * cp:2 * cp + 2, (jc % 4) * P:(jc % 4 + 1) * P],
                    wv_s[:, 2 * cp:2 * cp + 2, :],
                    start=(cp == 0), stop=(cp == CP - 1), perf_mode=DR,
                )
            if jc % 2 == 0:
                nc.vector.tensor_copy(VT[:, jc, :], ps[:])
            else:
                nc.scalar.copy(VT[:, jc, :], ps[:])

        # ---- attention + fused output projection -------------------------
        def emit_final(it, half, endgame=False):
            for cc in (half, half + 2):
                ps = psA.tile([P, 512], f32, tag="mm", name=f"psf_{it}_{cc}")
                for cp in range(CP):
                    nc.tensor.matmul(
                        ps[:],
                        wo_s[:, 2 * cp:2 * cp + 2, cc * P:(cc + 1) * P],
                        Ot[:, 2 * cp:2 * cp + 2, it * 512:(it + 1) * 512],
                        start=(cp == 0), stop=(cp == CP - 1), perf_mode=DR,
                    )
                ft = fpl.tile([P, 512], bf16, tag="f", name=f"ft_{it}_{cc}")
                if endgame and cc >= 2:
                    nc.scalar.copy(ft[:], ps[:])
                else:
                    nc.vector.tensor_copy(ft[:], ps[:])
                eng = nc.sync if cc % 2 == 0 else nc.gpsimd
                eng.dma_start(out[:, it, cc, :], ft[:])

        pend_evac = None
        for it in range(ITQ):
            l_ps = psS.tile([P, 512], f32, tag="l", name=f"l_ps_{it}")
            o_ps = [
                psO.tile([P, 512], f32, tag="o", name=f"o_ps_{it}_{cc}")
                for cc in range(CO)
            ]

            def emit_avl(jp, pt, l_ps=l_ps, o_ps=o_ps):
                nc.tensor.matmul(
                    l_ps[:], ones8[:], pt[:],
                    start=(jp == 0), stop=(jp == JPR - 1), perf_mode=DR,
                )
                for cc in range(CO):
                    nc.tensor.matmul(
                        o_ps[cc][:],
                        VT[:, 2 * jp:2 * jp + 2, cc * P:(cc + 1) * P],
                        pt[:],
                        start=(jp == 0), stop=(jp == JPR - 1), perf_mode=DR,
                    )

            # AV lags 2 pairs at slab start (hides prev-slab PSUM evac), 1 after
            prev = []
            for jp in range(JPR):
                if jp == 0 and pend_evac is not None:
                    pend_evac()     # prev slab's PSUM evac, first thing
                if jp == 3 and it > 0:
                    emit_final(it - 1, 0)   # overlap prev slab's out-proj
                if jp == 5 and it > 0:
                    emit_final(it - 1, 1)
                pt = wrk.tile(
                    [P, 2, 512], f8, tag="pt", bufs=4, name=f"pt_{it}_{jp}"
                )
                for t in range(2):
                    jc = 2 * jp + t
                    st = psA.tile([P, 512], f32, tag="mm", name=f"st_{it}_{jc}")
                    for cp in range(CP):
                        nc.tensor.matmul(
                            st[:],
                            Kt[:, 2 * cp:2 * cp + 2, jc * P:(jc + 1) * P],
                            Qt[:, 2 * cp:2 * cp + 2, it * 512:(it + 1) * 512],
                            start=(cp == 0), stop=(cp == CP - 1), perf_mode=DR,
                        )
                    nc.scalar.activation(
                        out=pt[:, t, :], in_=st[:], func=AF.Exp,
                        bias=csh_s[:], scale=SM_SCALE,
                    )
                prev.append((jp, pt))
                lag = 2 if jp < 4 else 1
                while len(prev) > lag:
                    emit_avl(*prev.pop(0))
            for pr in prev:
                emit_avl(*pr)

            def _evac(it=it, l_ps=l_ps, o_ps=o_ps, endgame=(it == ITQ - 1)):
                lt = wrk.tile([1, 512], f32, tag="lt", name=f"lt_{it}")
                nc.vector.tensor_copy(lt[:], l_ps[0:1, :])
                nc.sync.dma_start(lout[it:it + 1, :], lt[:])
                for cc in range(CO):
                    if endgame and cc % 2 == 1:
                        nc.scalar.copy(
                            Ot[:, cc, it * 512:(it + 1) * 512], o_ps[cc][:]
                        )
                    else:
                        nc.vector.tensor_copy(
                            Ot[:, cc, it * 512:(it + 1) * 512], o_ps[cc][:]
                        )
            pend_evac = _evac
        pend_evac()
        emit_final(ITQ - 1, 0, endgame=True)
        emit_final(ITQ - 1, 1, endgame=True)

    nc.compile()
    return nc


def _get_program():
    if "nc" not in _CACHE:
        _CACHE["nc"] = _build_program()
    return _CACHE["nc"]


def _tile_cp(a, dtype):
    """[C, M] -> [P, CO, M] with c = co*128 + p."""
    m = a.shape[1]
    return np.ascontiguousarray(
        a.reshape(CO, P, m).transpose(1, 0, 2).astype(dtype)
    )


def _tile_c(v):
    """[C] -> [P, CO] with c = co*128 + p."""
    return np.ascontiguousarray(v.reshape(CO, P).T, dtype=np.float32)


def _blockmajor(xt, nblk):
    """[P, CO, nblk*512] -> [P, nblk, CO, 512] contiguous."""
    return np.ascontiguousarray(
        xt.reshape(P, CO, nblk, 512).transpose(0, 2, 1, 3)
    )


def _host_prep(x, gamma, beta, wq, bq, wk, bk, wv, bv, wo, bo):
    import ml_dtypes

    bf16 = ml_dtypes.bfloat16
    f8 = ml_dtypes.float8_e4m3
    x = np.asarray(x, dtype=np.float32)
    b = x.shape[0]
    xv = x.reshape(b, C, N)

    # host GroupNorm (fp64 stats, f32 apply)
    gamma = np.asarray(gamma, np.float32)
    beta = np.asarray(beta, np.float32)
    xg = xv.reshape(b, G, GS * N)
    mean = xg.mean(axis=2, dtype=np.float64)                 # [b, G]
    var = xg.var(axis=2, dtype=np.float64)                   # [b, G]
    rstd = 1.0 / np.sqrt(var + EPS)
    mean_c = np.repeat(mean, GS, axis=1).astype(np.float32)  # [b, C]
    rstd_c = np.repeat(rstd, GS, axis=1).astype(np.float32)
    scl = rstd_c * gamma[None, :]
    shf = beta[None, :] - scl * mean_c
    h = xv * scl[:, :, None] + shf[:, :, None]               # [b, C, N] f32

    wqT = np.ascontiguousarray(np.asarray(wq, np.float32).T) * WS
    wkT = np.ascontiguousarray(np.asarray(wk, np.float32).T) * WS
    wvT = np.ascontiguousarray(np.asarray(wv, np.float32).T)
    woT = np.ascontiguousarray(np.asarray(wo, np.float32).T) * WS

    wq_t = _tile_cp(wqT, f8)
    wk_t = _tile_cp(wkT, f8)
    wv_t = _tile_cp(wvT, f8)
    wo_t = _tile_cp(woT, f8)
    bq_t = _tile_c(np.asarray(bq, np.float32) * WS)

    halves = {}
    for bi in range(b):
        ht = _tile_cp(h[bi], f8)                             # [P, CO, N]
        for hf in range(2):
            halves[(bi, hf)] = _blockmajor(
                ht[:, :, hf * NH:(hf + 1) * NH], KBLK
            )

    in_maps = []
    for core in range(8):
        bi, qh, kk = core // 4, (core // 2) % 2, core % 2
        in_maps.append({
            "hkv": halves[(bi, kk)], "hq": halves[(bi, qh)],
            "wk8": wk_t, "wv8": wv_t, "wq8": wq_t, "wot": wo_t,
            "bqb": bq_t,
        })
    return in_maps, b


def kernel(x, gamma, beta, wq, bq, wk, bk, wv, bv, wo, bo):
    from concourse.bass_utils import run_bass_kernel_spmd

    nc = _get_program()
    in_maps, b = _host_prep(x, gamma, beta, wq, bq, wk, bk, wv, bv, wo, bo)
    res = run_bass_kernel_spmd(nc, in_maps, core_ids=list(range(8)))

    x = np.asarray(x, dtype=np.float32)
    xv = x.reshape(b, C, N)
    bo_eff = (
        np.asarray(bo, np.float64)
        + np.asarray(wo, np.float64) @ np.asarray(bv, np.float64)
    )
    outp = np.empty((b, C, N), dtype=np.float32)
    for bi in range(b):
        for qh in range(2):
            ca = bi * 4 + qh * 2 + 0   # key-half 0
            cb = bi * 4 + qh * 2 + 1   # key-half 1
            fu = (
                res.results[ca]["out"].astype(np.float64)
                + res.results[cb]["out"].astype(np.float64)
            )  # [P, ITQ, CO, 512]
            l = (
                res.results[ca]["lout"].astype(np.float64)
                + res.results[cb]["lout"].astype(np.float64)
            ).reshape(NH)
            fu = fu.transpose(2, 0, 1, 3).reshape(C, NH)  # channel-major
            cols = slice(qh * NH, (qh + 1) * NH)
            outp[bi, :, cols] = (
                xv[bi][:, cols] + fu / (WS * l[None, :]) + bo_eff[:, None]
            )
    return outp.reshape(b, C, 16, 16, 16)


# revision 41
# speedup vs baseline: 1.2056x; 1.0137x over previous
"""AttnBlock3D (GroupNorm + single-head self-attention over 4096 voxels + residual)
for Trainium2, SPMD over 8 NeuronCores.

2D sharding: core = b*4 + q*2 + kk  (b batch, q query-half, kk key-half).

Host: GroupNorm (fp64 stats) and all bias folding happen on the host; each core
receives its 2048-column query half and key half of normalized h in fp8(e4m3),
plus fp8 Q/K/V weights prescaled by WS=16 and a bf16 output-projection weight.
K and V biases fold out exactly (k-bias is constant across keys so it cancels
in softmax; v-bias times sum-of-probs folds into the host-side constant
bo_eff = bo + wo @ bv).  Only bq remains on-device (scaled by WS).

Device (one NEFF, no collectives): all of Q/K/V projection, logits and A@V run
as fp8 DoubleRow matmuls (256-deep contraction per instruction); probs are fp8
via exp(S*scale - CSH) on ScalarE (the constant shift cancels in softmax and
keeps fp8 probs < 240); the row-sum l rides a fp8 ones-matmul; out-projection
wo @ o_u runs in bf16; unnormalized F_u (bf16) and l (f32) go back to the host.
Host combine: out = x + (F_u^k0 + F_u^k1) / (WS * (l^k0 + l^k1)) + bo_eff.
"""

import sys

if "/opt/trn_rl_repo" not in sys.path:
    sys.path.insert(0, "/opt/trn_rl_repo")

import numpy as np

P = 128
C = 512
CO = C // P          # 4 channel chunks
CP = CO // 2         # 2 channel-chunk pairs (DoubleRow)
N = 4096             # spatial size (16^3)
NH = N // 2          # 2048 local keys / queries
KBLK = NH // 512     # 4 key blocks
ITQ = NH // 512      # 4 query slabs
JCK = NH // P        # 16 local key chunks
JPR = JCK // 2       # 8 key chunk pairs (DoubleRow)
G = 32               # groups
GS = C // G          # 16 channels per group
EPS = 1e-6
WS = 16.0            # fp8 weight prescale (folded into exp scale + host combine)
CSH = 5.0            # exp shift: probs = exp(S*scale - CSH), cancels in softmax
SM_SCALE = float(C) ** -0.5 / (WS * WS)

_CACHE = {}


def _build_program():
    import concourse.bass as bass
    import concourse.tile as tile
    import concourse.mybir as mybir
    from concourse import bacc
    from contextlib import ExitStack

    f32 = mybir.dt.float32
    bf16 = mybir.dt.bfloat16
    f8 = mybir.dt.float8e4
    AF = mybir.ActivationFunctionType
    DR = mybir.MatmulPerfMode.DoubleRow

    nc = bacc.Bacc("TRN2", target_bir_lowering=False)

    hkv = nc.dram_tensor("hkv", [P, KBLK, CO, 512], f8, kind="ExternalInput")
    hq = nc.dram_tensor("hq", [P, ITQ, CO, 512], f8, kind="ExternalInput")
    wk8 = nc.dram_tensor("wk8", [P, CO, C], f8, kind="ExternalInput")
    wv8 = nc.dram_tensor("wv8", [P, CO, C], f8, kind="ExternalInput")
    wq8 = nc.dram_tensor("wq8", [P, CO, C], f8, kind="ExternalInput")
    wot = nc.dram_tensor("wot", [P, CO, C], f8, kind="ExternalInput")
    bqb = nc.dram_tensor("bqb", [P, CO], f32, kind="ExternalInput")
    out = nc.dram_tensor("out", [P, ITQ, CO, 512], bf16, kind="ExternalOutput")
    lout = nc.dram_tensor("lout", [ITQ, 512], f32, kind="ExternalOutput")
    warm = nc.dram_tensor("warm", [1, 1], f32, kind="ExternalOutput")

    with ExitStack() as ctx:
        tc = ctx.enter_context(tile.TileContext(nc))
        big = ctx.enter_context(tc.tile_pool(name="big", bufs=1))
        wrk = ctx.enter_context(tc.tile_pool(name="wrk", bufs=3))
        fpl = ctx.enter_context(tc.tile_pool(name="fpl", bufs=2))
        psA = ctx.enter_context(tc.tile_pool(name="psA", bufs=3, space="PSUM"))
        psO = ctx.enter_context(tc.tile_pool(name="psO", bufs=4, space="PSUM"))
        psS = ctx.enter_context(tc.tile_pool(name="psS", bufs=1, space="PSUM"))

        # ---- persistent SBUF tiles -------------------------------------
        Hkv = big.tile([P, KBLK, CO, 512], f8)    # key-half h
        Hq = big.tile([P, ITQ, CO, 512], f8)      # query-half h
        Kt = big.tile([P, CO, NH], f8)            # k16[c, j]
        Qt = big.tile([P, CO, NH], f8)            # q16[c, i]
        VT = big.tile([P, JCK, C], f8)            # v16^T[j, c]
        Ot = big.tile([P, CO, NH], f8)            # o_u (true scale)
        wk_s = big.tile([P, CO, C], f8)
        wv_s = big.tile([P, CO, C], f8)
        wq_s = big.tile([P, CO, C], f8)
        wo_s = big.tile([P, CO, C], f8)
        bq_s = big.tile([P, CO], f32)
        ones8 = big.tile([P, 2, P], f8)
        junk_rhs = big.tile([P, 2, 512], f8)
        csh_s = big.tile([P, 1], f32)
        nc.vector.memset(ones8, 1.0)
        nc.gpsimd.memset(junk_rhs, 0.0)
        nc.vector.memset(csh_s, -CSH)

        # PE warm-up while input DMAs stream (opens the clock gate and
        # exercises the fp8 DoubleRow path). The accumulation chain plus the
        # DMA'd sample keep walrus from dead-code-eliminating it.
        junk_ps = psS.tile([P, 512], f32, tag="l", name="junk_ps")
        NWARM = 12
        for j in range(NWARM):
            sl = slice((j % 2) * 256, (j % 2) * 256 + 256)
            nc.tensor.matmul(
                junk_ps[0:32, sl], ones8[:, :, (j % 4) * 32:(j % 4) * 32 + 32],
                junk_rhs[:, :, sl],
                start=(j < 2), stop=(j >= NWARM - 2), perf_mode=DR,
            )
        wrm = big.tile([1, 1], f32)
        nc.vector.tensor_copy(wrm[:], junk_ps[0:1, 0:1])
        nc.sync.dma_start(warm[:, :], wrm[:])

        # ---- input DMAs: interleaved across the 3 DMA-capable queues in
        # consumption order (K path first, then Q path, then V/O weights) so
        # arrival order under shared HBM bandwidth matches the compute order.
        nc.sync.dma_start(wk_s[:, 0:2, :], wk8[:, 0:2, :])
        nc.gpsimd.dma_start(wk_s[:, 2:4, :], wk8[:, 2:4, :])
        nc.scalar.dma_start(wv_s[:], wv8[:, :, :])
        nc.sync.dma_start(Hkv[:, 0, :, :], hkv[:, 0, :, :])
        nc.gpsimd.dma_start(Hkv[:, 1, :, :], hkv[:, 1, :, :])
        nc.scalar.dma_start(Hkv[:, 2, :, :], hkv[:, 2, :, :])
        nc.sync.dma_start(Hkv[:, 3, :, :], hkv[:, 3, :, :])
        nc.gpsimd.dma_start(wq_s[:, 0:2, :], wq8[:, 0:2, :])
        nc.scalar.dma_start(wq_s[:, 2:4, :], wq8[:, 2:4, :])
        nc.sync.dma_start(bq_s[:], bqb[:, :])
        nc.gpsimd.dma_start(Hq[:, 0, :, :], hq[:, 0, :, :])
        nc.scalar.dma_start(Hq[:, 1, :, :], hq[:, 1, :, :])
        nc.sync.dma_start(Hq[:, 2, :, :], hq[:, 2, :, :])
        nc.gpsimd.dma_start(Hq[:, 3, :, :], hq[:, 3, :, :])
        nc.scalar.dma_start(wo_s[:], wot[:, :, :])

        # ---- projections (fp8 DoubleRow: 256-deep contraction) ----------
        # K and V^T interleaved per 512-col block so compute tracks the
        # arrival order of the Hkv block DMAs.
        for blk in range(KBLK):
            for cc in range(CO):
                ps = psA.tile([P, 512], f32, tag="mm", name=f"psk_{blk}_{cc}")
                for cp in range(CP):
                    nc.tensor.matmul(
                        ps[:],
                        wk_s[:, 2 * cp:2 * cp + 2, cc * P:(cc + 1) * P],
                        Hkv[:, blk, 2 * cp:2 * cp + 2, :],
                        start=(cp == 0), stop=(cp == CP - 1), perf_mode=DR,
                    )
                if cc % 2 == 0:
                    nc.vector.tensor_copy(
                        Kt[:, cc, blk * 512:(blk + 1) * 512], ps[:]
                    )
                else:
                    nc.scalar.copy(Kt[:, cc, blk * 512:(blk + 1) * 512], ps[:])
            for jc in range(4 * blk, 4 * blk + 4):
                ps = psA.tile([P, 512], f32, tag="mm", name=f"psv_{jc}")
                for cp in range(CP):
                    nc.tensor.matmul(
                        ps[:],
                        Hkv[:, blk, 2 * cp:2 * cp + 2,
                            (jc % 4) * P:(jc % 4 + 1) * P],
                        wv_s[:, 2 * cp:2 * cp + 2, :],
                        start=(cp == 0), stop=(cp == CP - 1), perf_mode=DR,
                    )
                if jc % 2 == 0:
                    nc.vector.tensor_copy(VT[:, jc, :], ps[:])
                else:
                    nc.scalar.copy(VT[:, jc, :], ps[:])

        # Q: q16[cc, it] = sum_ci wq16[cc, ci] hq[ci, it] + 16*bq
        for it in range(ITQ):
            for cc in range(CO):
                ps = psA.tile([P, 512], f32, tag="mm", name=f"psq_{it}_{cc}")
                for cp in range(CP):
                    nc.tensor.matmul(
                        ps[:],
                        wq_s[:, 2 * cp:2 * cp + 2, cc * P:(cc + 1) * P],
                        Hq[:, it, 2 * cp:2 * cp + 2, :],
                        start=(cp == 0), stop=(cp == CP - 1), perf_mode=DR,
                    )
                if cc % 2 == 0:
                    nc.vector.tensor_scalar_add(
                        Qt[:, cc, it * 512:(it + 1) * 512], ps[:],
                        bq_s[:, cc:cc + 1],
                    )
                else:
                    nc.scalar.activation(
                        out=Qt[:, cc, it * 512:(it + 1) * 512], in_=ps[:],
                        func=AF.Identity, bias=bq_s[:, cc:cc + 1], scale=1.0,
                    )

        # ---- attention + fused output projection -------------------------
        def emit_final(it, half, endgame=False):
            for cc in (half, half + 2):
                ps = psA.tile([P, 512], f32, tag="mm", name=f"psf_{it}_{cc}")
                for cp in range(CP):
                    nc.tensor.matmul(
                        ps[:],
                        wo_s[:, 2 * cp:2 * cp + 2, cc * P:(cc + 1) * P],
                        Ot[:, 2 * cp:2 * cp + 2, it * 512:(it + 1) * 512],
                        start=(cp == 0), stop=(cp == CP - 1), perf_mode=DR,
                    )
                ft = fpl.tile([P, 512], bf16, tag="f", name=f"ft_{it}_{cc}")
                if endgame and cc >= 2:
                    nc.scalar.copy(ft[:], ps[:])
                else:
                    nc.vector.tensor_copy(ft[:], ps[:])
                if endgame:
                    eng = nc.sync if cc < 2 else nc.scalar
                else:
                    eng = nc.sync if cc % 2 == 0 else nc.gpsimd
                eng.dma_start(out[:, it, cc, :], ft[:])

        pend_evac = None
        for it in range(ITQ):
            l_ps = psS.tile([P, 512], f32, tag="l", name=f"l_ps_{it}")
            o_ps = [
                psO.tile([P, 512], f32, tag="o", name=f"o_ps_{it}_{cc}")
                for cc in range(CO)
            ]

            def emit_avl(jp, pt, l_ps=l_ps, o_ps=o_ps):
                nc.tensor.matmul(
                    l_ps[:], ones8[:], pt[:],
                    start=(jp == 0), stop=(jp == JPR - 1), perf_mode=DR,
                )
                for cc in range(CO):
                    nc.tensor.matmul(
                        o_ps[cc][:],
                        VT[:, 2 * jp:2 * jp + 2, cc * P:(cc + 1) * P],
                        pt[:],
                        start=(jp == 0), stop=(jp == JPR - 1), perf_mode=DR,
                    )

            # AV lags 2 pairs at slab start (hides prev-slab PSUM evac), 1 after
            prev = []
            for jp in range(JPR):
                if jp == 0 and pend_evac is not None:
                    pend_evac()     # prev slab's PSUM evac, first thing
                if jp == 3 and it > 0:
                    emit_final(it - 1, 0)   # overlap prev slab's out-proj
                if jp == 5 and it > 0:
                    emit_final(it - 1, 1)
                pt = wrk.tile(
                    [P, 2, 512], f8, tag="pt", bufs=4, name=f"pt_{it}_{jp}"
                )
                for t in range(2):
                    jc = 2 * jp + t
                    st = psA.tile([P, 512], f32, tag="mm", name=f"st_{it}_{jc}")
                    for cp in range(CP):
                        nc.tensor.matmul(
                            st[:],
                            Kt[:, 2 * cp:2 * cp + 2, jc * P:(jc + 1) * P],
                            Qt[:, 2 * cp:2 * cp + 2, it * 512:(it + 1) * 512],
                            start=(cp == 0), stop=(cp == CP - 1), perf_mode=DR,
                        )
                    nc.scalar.activation(
                        out=pt[:, t, :], in_=st[:], func=AF.Exp,
                        bias=csh_s[:], scale=SM_SCALE,
                    )
                prev.append((jp, pt))
                lag = 2 if jp < 4 else 1
                while len(prev) > lag:
                    emit_avl(*prev.pop(0))
            for pr in prev:
                emit_avl(*pr)

            def _evac(it=it, l_ps=l_ps, o_ps=o_ps, endgame=(it == ITQ - 1)):
                lt = wrk.tile([1, 512], f32, tag="lt", name=f"lt_{it}")
                nc.vector.tensor_copy(lt[:], l_ps[0:1, :])
                nc.sync.dma_start(lout[it:it + 1, :], lt[:])
                for cc in range(CO):
                    if endgame and cc % 2 == 1:
                        nc.scalar.copy(
                            Ot[:, cc, it * 512:(it + 1) * 512], o_ps[cc][:]
                        )
                    else:
                        nc.vector.tensor_copy(
                            Ot[:, cc, it * 512:(it + 1) * 512], o_ps[cc][:]
                        )
            pend_evac = _evac
        pend_evac()
        emit_final(ITQ - 1, 0, endgame=True)
        emit_final(ITQ - 1, 1, endgame=True)

    nc.compile()
    return nc


def _get_program():
    if "nc" not in _CACHE:
        _CACHE["nc"] = _build_program()
    return _CACHE["nc"]


def _tile_cp(a, dtype):
    """[C, M] -> [P, CO, M] with c = co*128 + p."""
    m = a.shape[1]
    return np.ascontiguousarray(
        a.reshape(CO, P, m).transpose(1, 0, 2).astype(dtype)
    )


def _tile_c(v):
    """[C] -> [P, CO] with c = co*128 + p."""
    return np.ascontiguousarray(v.reshape(CO, P).T, dtype=np.float32)


def _blockmajor(xt, nblk):
    """[P, CO, nblk*512] -> [P, nblk, CO, 512] contiguous."""
    return np.ascontiguousarray(
        xt.reshape(P, CO, nblk, 512).transpose(0, 2, 1, 3)
    )


def _host_prep(x, gamma, beta, wq, bq, wk, bk, wv, bv, wo, bo):
    import ml_dtypes

    bf16 = ml_dtypes.bfloat16
    f8 = ml_dtypes.float8_e4m3
    x = np.asarray(x, dtype=np.float32)
    b = x.shape[0]
    xv = x.reshape(b, C, N)

    # host GroupNorm (fp64 stats, f32 apply)
    gamma = np.asarray(gamma, np.float32)
    beta = np.asarray(beta, np.float32)
    xg = xv.reshape(b, G, GS * N)
    mean = xg.mean(axis=2, dtype=np.float64)                 # [b, G]
    var = xg.var(axis=2, dtype=np.float64)                   # [b, G]
    rstd = 1.0 / np.sqrt(var + EPS)
    mean_c = np.repeat(mean, GS, axis=1).astype(np.float32)  # [b, C]
    rstd_c = np.repeat(rstd, GS, axis=1).astype(np.float32)
    scl = rstd_c * gamma[None, :]
    shf = beta[None, :] - scl * mean_c
    h = xv * scl[:, :, None] + shf[:, :, None]               # [b, C, N] f32

    wqT = np.ascontiguousarray(np.asarray(wq, np.float32).T) * WS
    wkT = np.ascontiguousarray(np.asarray(wk, np.float32).T) * WS
    wvT = np.ascontiguousarray(np.asarray(wv, np.float32).T)
    woT = np.ascontiguousarray(np.asarray(wo, np.float32).T) * WS

    wq_t = _tile_cp(wqT, f8)
    wk_t = _tile_cp(wkT, f8)
    wv_t = _tile_cp(wvT, f8)
    wo_t = _tile_cp(woT, f8)
    bq_t = _tile_c(np.asarray(bq, np.float32) * WS)

    halves = {}
    for bi in range(b):
        ht = _tile_cp(h[bi], f8)                             # [P, CO, N]
        for hf in range(2):
            halves[(bi, hf)] = _blockmajor(
                ht[:, :, hf * NH:(hf + 1) * NH], KBLK
            )

    in_maps = []
    for core in range(8):
        bi, qh, kk = core // 4, (core // 2) % 2, core % 2
        in_maps.append({
            "hkv": halves[(bi, kk)], "hq": halves[(bi, qh)],
            "wk8": wk_t, "wv8": wv_t, "wq8": wq_t, "wot": wo_t,
            "bqb": bq_t,
        })
    return in_maps, b


def kernel(x, gamma, beta, wq, bq, wk, bk, wv, bv, wo, bo):
    from concourse.bass_utils import run_bass_kernel_spmd

    nc = _get_program()
    in_maps, b = _host_prep(x, gamma, beta, wq, bq, wk, bk, wv, bv, wo, bo)
    res = run_bass_kernel_spmd(nc, in_maps, core_ids=list(range(8)))

    x = np.asarray(x, dtype=np.float32)
    xv = x.reshape(b, C, N)
    bo_eff = (
        np.asarray(bo, np.float64)
        + np.asarray(wo, np.float64) @ np.asarray(bv, np.float64)
    )
    outp = np.empty((b, C, N), dtype=np.float32)
    for bi in range(b):
        for qh in range(2):
            ca = bi * 4 + qh * 2 + 0   # key-half 0
            cb = bi * 4 + qh * 2 + 1   # key-half 1
            fu = (
                res.results[ca]["out"].astype(np.float64)
                + res.results[cb]["out"].astype(np.float64)
            )  # [P, ITQ, CO, 512]
            l = (
                res.results[ca]["lout"].astype(np.float64)
                + res.results[cb]["lout"].astype(np.float64)
            ).reshape(NH)
            fu = fu.transpose(2, 0, 1, 3).reshape(C, NH)  # channel-major
            cols = slice(qh * NH, (qh + 1) * NH)
            outp[bi, :, cols] = (
                xv[bi][:, cols] + fu / (WS * l[None, :]) + bo_eff[:, None]
            )
    return outp.reshape(b, C, 16, 16, 16)
